# revision 8
# baseline (speedup 1.0000x reference)
"""Trainium2 Bass kernel for nn_ModalityAlignmentModel_2 (topk_masking).

Computation (see reference):
  h = relu(conv2d(x, conv_w) + conv_b)           (1,16,628,512) -> (16, 321536)
  audio = h @ wa.T + ba                          (16, 1024)
  temb  = text_features @ wt.T + bt              (8192, 256)
  scores[k] = audio[:, 256k:256k+256] @ temb.T   (4, 16, 8192)
  probs = softmax(scores, -1); flat per k; loss from top5/bot5 probs.

Sharding (8 cores):
  - Contraction dim of the big matmul (321536) is sharded: core c computes a
    partial (16, 1024) audio embedding from its 40192-slice of h and of wa^T,
    then AllReduce(add).  wa is pre-transposed on host so the contraction dim
    lands on SBUF partitions with fully contiguous DMA.
  - text_features batch dim (8192) is data-parallel: core c computes
    temb[c*1024:(c+1)*1024] (as temb^T (256, 1024)); host concatenates.
  - conv is computed per-core for its own 40192 spatial slice via a host-built
    im2col (10 x 40192, row 9 = ones to fold conv_b into the matmul).
  - softmax over the full 8192 via a tiny AllGather of per-core (max, sum)
    stats; global top-5/bot-5 via per-row top-8 (vector.max), an AllGather of
    per-core candidates, and a final top-8 per sub-embedding.
"""
import sys

import numpy as np

sys.path.insert(0, "/opt/trn_rl_repo")

import concourse.bacc as bacc
import concourse.bass as bass
import concourse.mybir as mybir
import concourse.tile as tile
from concourse import bass_utils, masks

N_CORES = 8
AUDIO_DIM = 1024
TEXT_DIM = 256
N_SUB = 4
K_TOP = 5
H, W = 628, 512
KTOT = H * W            # 321536
KSH = KTOT // N_CORES   # 40192 contraction elements per core
NCHUNK = KSH // 128     # 314 k-chunks of 128
NPAIR = NCHUNK // 2     # 157 pairs (1MB wa DMA each)
NLOC = 8192 // N_CORES  # 1024 text rows per core
TD = 3072               # text feature dim
F32 = mybir.dt.float32
F32R = mybir.dt.float32r
AF = mybir.ActivationFunctionType
AX = mybir.AxisListType

# pairs at which a text d-chunk (of 24) is processed, spread over the wa loop
_TEXT_AT = {t * 6 + 3: t for t in range(24)}

# dtype used for the big wa matmul inputs (f32r = 4x faster PE, same bits)
MAIN_MM_DT = F32
DEBUG_OUT = False
CONV_PREFETCH = 2  # pairs of conv lookahead


def _build():
    nc = bacc.Bacc("TRN2", target_bir_lowering=False, debug=False,
                   enable_asserts=True, num_devices=N_CORES)

    imcol_in = nc.dram_tensor("imcol", [10, KSH], F32, kind="ExternalInput")
    wc_in = nc.dram_tensor("wc", [10, 16], F32, kind="ExternalInput")
    waT_in = nc.dram_tensor("waT", [KSH, AUDIO_DIM], F32, kind="ExternalInput")
    textT_in = nc.dram_tensor("textT", [TD, NLOC], F32, kind="ExternalInput")
    wtT_in = nc.dram_tensor("wtT", [TD, TEXT_DIM], F32, kind="ExternalInput")
    ba_in = nc.dram_tensor("ba2", [1, AUDIO_DIM], F32, kind="ExternalInput")
    btT_in = nc.dram_tensor("btT", [128, 2], F32, kind="ExternalInput")

    audio_out = nc.dram_tensor("audio_out", [16, AUDIO_DIM], F32, kind="ExternalOutput")
    temb_out = nc.dram_tensor("temb_out", [TEXT_DIM, NLOC], F32, kind="ExternalOutput")
    loss_out = nc.dram_tensor("loss_out", [1, 1], F32, kind="ExternalOutput")
    dbg = {}
    if DEBUG_OUT:
        for nm, shp in [("d_scores", [128, NLOC]), ("d_probs", [128, NLOC]),
                        ("d_top8", [128, 8]), ("d_bot8", [128, 8]),
                        ("d_cand", [1, 1024]), ("d_ag2", [N_CORES, 1024]),
                        ("d_gtop", [4, 1024]), ("d_gbot", [4, 1024]),
                        ("d_gt8", [4, 8]), ("d_gb8", [4, 8]),
                        ("d_lt", [4, 10]), ("d_sp", [4, 10]),
                        ("d_ls4", [4, 1]), ("d_allst", [128, 16])]:
            dbg[nm] = nc.dram_tensor(nm, shp, F32, kind="ExternalOutput")

    def mdt(ap):
        return ap.bitcast(MAIN_MM_DT) if MAIN_MM_DT != F32 else ap

    with tile.TileContext(nc) as tc:
        with tc.tile_pool(name="const", bufs=1) as cpool, \
             tc.tile_pool(name="ic", bufs=3) as icpool, \
             tc.tile_pool(name="wa", bufs=6) as wapool, \
             tc.tile_pool(name="ht", bufs=2 * (CONV_PREFETCH + 2)) as htpool, \
             tc.tile_pool(name="tx", bufs=3) as txpool, \
             tc.tile_pool(name="big", bufs=1) as bigpool, \
             tc.tile_pool(name="dram", bufs=1, space="DRAM") as dram, \
             tc.tile_pool(name="pa", bufs=1, space="PSUM") as pa, \
             tc.tile_pool(name="pc", bufs=2, space="PSUM") as pc:

            # ---- constants ----
            wc_sb = cpool.tile([10, 16], F32)
            nc.gpsimd.dma_start(wc_sb[:], wc_in[:])
            wtT_sb = cpool.tile([128, 24 * TEXT_DIM], F32)
            nc.gpsimd.dma_start(
                wtT_sb[:].rearrange("p (i o) -> p i o", i=24),
                wtT_in[:].rearrange("(i p) o -> p i o", p=128))
            btT_sb = cpool.tile([128, 2], F32)
            nc.gpsimd.dma_start(btT_sb[:], btT_in[:])
            ba_sb = cpool.tile([1, AUDIO_DIM], F32)
            nc.gpsimd.dma_start(ba_sb[:], ba_in[:])
            ident = cpool.tile([16, 16], F32)
            masks.make_identity(nc, ident[:])
            ones16 = cpool.tile([1, 16], F32)
            nc.vector.memset(ones16[:], 1.0)

            temb_sb = [bigpool.tile([128, NLOC], F32, tag=f"temb{j}", name=f"temb{j}")
                       for j in range(2)]

            audio_ps = pa.tile([16, AUDIO_DIM], F32)

            ic_tiles = {}  # block -> tile
            ht_tiles = {}  # chunk -> tile

            def load_ic_block(b):
                cols = min(2048, KSH - b * 2048)
                t = icpool.tile([10, 2048], F32)
                nc.gpsimd.dma_start(t[0:10, 0:cols],
                                    imcol_in[:, b * 2048:b * 2048 + cols])
                ic_tiles[b] = t

            def conv_chunk(i):
                """im2col matmul for k-chunk i -> ht_tiles[i] ([128,16] sbuf)."""
                b, off = divmod(i * 128, 2048)
                if b not in ic_tiles:
                    load_ic_block(b)
                cps = pc.tile([128, 16], F32, tag="small")
                nc.tensor.matmul(cps[:], ic_tiles[b][0:10, off:off + 128],
                                 wc_sb[:], start=True, stop=True)
                ht = htpool.tile([128, 16], F32)
                nc.scalar.activation(ht[:], cps[:], AF.Relu)
                ht_tiles[i] = ht

            with tc.tile_pool(name="pt", bufs=1, space="PSUM") as pt:
                text_ps = [pt.tile([128, 512], F32, tag=f"pt{j}{t}", name=f"pt{j}{t}")
                           for j in range(2) for t in range(2)]

                def text_chunk(ti):
                    tx = txpool.tile([128, NLOC], F32)
                    nc.gpsimd.dma_start(tx[:], textT_in[ti * 128:(ti + 1) * 128, :])
                    for j in range(2):
                        for t in range(2):
                            nc.tensor.matmul(
                                text_ps[2 * j + t][:],
                                wtT_sb[:, ti * TEXT_DIM + j * 128:
                                       ti * TEXT_DIM + j * 128 + 128],
                                tx[:, t * 512:(t + 1) * 512],
                                start=(ti == 0), stop=(ti == 23))

                # conv prologue
                for i in range(2 * CONV_PREFETCH):
                    conv_chunk(i)

                # ---- main loop: stream waT, accumulate audio partial ----
                for p in range(NPAIR):
                    wa_t = wapool.tile([128, 2048], F32)
                    eng = nc.sync if (p % 2 == 0) else nc.scalar
                    eng.dma_start(
                        wa_t[:].rearrange("p (c o) -> p c o", c=2),
                        waT_in[p * 256:(p + 1) * 256, :]
                        .rearrange("(c p) o -> p c o", p=128))
                    # conv lookahead
                    base = (p + CONV_PREFETCH) * 2
                    for s in (0, 1):
                        if base + s < NCHUNK:
                            conv_chunk(base + s)
                    # audio matmuls for this pair
                    for s in (0, 1):
                        i = 2 * p + s
                        ht = ht_tiles.pop(i)
                        for half in (0, 1):
                            nc.tensor.matmul(
                                audio_ps[:, half * 512:(half + 1) * 512],
                                mdt(ht[:]),
                                mdt(wa_t[:, s * 1024 + half * 512:
                                         s * 1024 + (half + 1) * 512]),
                                start=(i == 0), stop=(i == NCHUNK - 1))
                    if p in _TEXT_AT:
                        text_chunk(_TEXT_AT[p])

                # ---- text epilogue: bias + copy out ----
                for j in range(2):
                    for t in range(2):
                        nc.scalar.activation(
                            temb_sb[j][:, t * 512:(t + 1) * 512],
                            text_ps[2 * j + t][:], AF.Identity,
                            bias=btT_sb[:, j:j + 1])
                    nc.sync.dma_start(
                        temb_out[j * 128:(j + 1) * 128, :], temb_sb[j][:])

            # ---- audio epilogue: AllReduce + bias + transpose ----
            audio_sb = bigpool.tile([16, AUDIO_DIM], F32)
            nc.vector.tensor_copy(audio_sb[:], audio_ps[:])
            ar_i = dram.tile([16, AUDIO_DIM], F32)
            ar_o = dram.tile([16, AUDIO_DIM], F32)
            nc.gpsimd.dma_start(ar_i[:], audio_sb[:])
            nc.gpsimd.collective_compute(
                "AllReduce", mybir.AluOpType.add,
                replica_groups=[list(range(N_CORES))],
                ins=[ar_i.opt()], outs=[ar_o.opt()])
            audio_red = bigpool.tile([16, AUDIO_DIM], F32)
            nc.gpsimd.dma_start(audio_red[:], ar_o[:])

            with tc.tile_pool(name="pe", bufs=2, space="PSUM") as pe:
                bb_ps = pe.tile([16, AUDIO_DIM], F32, tag="wide")
                for half in (0, 1):
                    nc.tensor.matmul(bb_ps[:, half * 512:(half + 1) * 512],
                                     ones16[:],
                                     ba_sb[:, half * 512:(half + 1) * 512],
                                     start=True, stop=True)
                audio_f = bigpool.tile([16, AUDIO_DIM], F32)
                nc.vector.tensor_add(audio_f[:], audio_red[:], bb_ps[:])
                nc.sync.dma_start(audio_out[:], audio_f[:])

                audioT = bigpool.tile([128, 128], F32)
                for j in range(8):
                    tp = pc.tile([128, 16], F32, tag="small")
                    nc.tensor.transpose(tp[:], audio_f[:, j * 128:(j + 1) * 128],
                                        ident[:])
                    nc.vector.tensor_copy(audioT[:, j * 16:(j + 1) * 16], tp[:])

                # ---- scores: (4, 16, 1024_local) ----
                # row layout: sub k occupies partitions [k*32, k*32+16)
                # (32-aligned engine base-partition requirement); rest zeroed.
                scores_sb = bigpool.tile([128, NLOC], F32)
                nc.vector.memset(scores_sb[:], 0.0)
                for k in range(N_SUB):
                    sc = pe.tile([16, NLOC], F32, tag="wide")
                    for h in (0, 1):
                        for t in (0, 1):
                            nc.tensor.matmul(
                                sc[:, t * 512:(t + 1) * 512],
                                audioT[:, (2 * k + h) * 16:(2 * k + h + 1) * 16],
                                temb_sb[h][:, t * 512:(t + 1) * 512],
                                start=(h == 0), stop=(h == 1))
                    nc.vector.tensor_copy(
                        scores_sb[k * 32:k * 32 + 16, :], sc[:])

            # ---- softmax with global stats ----
            m_loc = bigpool.tile([128, 1], F32)
            nc.vector.reduce_max(m_loc[:], scores_sb[:], axis=AX.X)
            negm = bigpool.tile([128, 1], F32)
            nc.vector.tensor_scalar_mul(negm[:], m_loc[:], -1.0)
            e_loc = bigpool.tile([128, NLOC], F32)
            z_loc = bigpool.tile([128, 1], F32)
            nc.scalar.activation(e_loc[:], scores_sb[:], AF.Exp,
                                 bias=negm[:], accum_out=z_loc[:])

            stats = bigpool.tile([128, 2], F32)
            nc.vector.tensor_copy(stats[:, 0:1], m_loc[:])
            nc.vector.tensor_copy(stats[:, 1:2], z_loc[:])
            ag1_i = dram.tile([128, 2], F32)
            ag1_o = dram.tile([128 * N_CORES, 2], F32)
            nc.gpsimd.dma_start(ag1_i[:], stats[:])
            nc.gpsimd.collective_compute(
                "AllGather", mybir.AluOpType.bypass,
                replica_groups=[list(range(N_CORES))],
                ins=[ag1_i.opt()], outs=[ag1_o.opt()])
            allst = bigpool.tile([128, 16], F32)
            nc.gpsimd.dma_start(
                allst[:].rearrange("r (c v) -> r c v", c=N_CORES),
                ag1_o[:].rearrange("(c r) v -> r c v", c=N_CORES))

            allst_v = allst[:].rearrange("r (c v) -> r c v", c=N_CORES)
            m_cols = allst_v[:, :, 0]
            z_cols = allst_v[:, :, 1]
            gmax = bigpool.tile([128, 1], F32)
            nc.vector.reduce_max(gmax[:], m_cols, axis=AX.X)
            neggm = bigpool.tile([128, 1], F32)
            nc.vector.tensor_scalar_mul(neggm[:], gmax[:], -1.0)
            t8 = bigpool.tile([128, N_CORES], F32)
            nc.scalar.activation(t8[:], m_cols, AF.Exp, bias=neggm[:])
            tz = bigpool.tile([128, N_CORES], F32)
            nc.vector.tensor_mul(tz[:], t8[:], z_cols)
            zg = bigpool.tile([128, 1], F32)
            nc.vector.reduce_sum(zg[:], tz[:], axis=AX.X)
            rz = bigpool.tile([128, 1], F32)
            nc.vector.reciprocal(rz[:], zg[:])

            probs = bigpool.tile([128, NLOC], F32)
            nc.scalar.activation(probs[:], scores_sb[:], AF.Exp, bias=neggm[:])
            nc.vector.tensor_scalar_mul(probs[:], probs[:], rz[:])
            negp = bigpool.tile([128, NLOC], F32)
            nc.vector.tensor_scalar_mul(negp[:], probs[:], -1.0)

            # ---- local top-8 / bot-8 per (k, b) row, then AllGather ----
            top8 = bigpool.tile([128, 8], F32)
            nc.vector.max(top8[:], probs[:])
            bot8n = bigpool.tile([128, 8], F32)
            nc.vector.max(bot8n[:], negp[:])
            cand = bigpool.tile([1, 1024], F32)
            for k in range(N_SUB):
                nc.sync.dma_start(cand[0:1, k * 128:(k + 1) * 128],
                                  top8[k * 32:k * 32 + 16, :])
                nc.sync.dma_start(cand[0:1, 512 + k * 128:512 + (k + 1) * 128],
                                  bot8n[k * 32:k * 32 + 16, :])
            ag2_i = dram.tile([1, 1024], F32)
            ag2_o = dram.tile([N_CORES, 1024], F32)
            nc.gpsimd.dma_start(ag2_i[:], cand[:])
            nc.gpsimd.collective_compute(
                "AllGather", mybir.AluOpType.bypass,
                replica_groups=[list(range(N_CORES))],
                ins=[ag2_i.opt()], outs=[ag2_o.opt()])
            gtop = bigpool.tile([4, 1024], F32)
            nc.gpsimd.dma_start(
                gtop[:].rearrange("k (c j) -> k c j", c=N_CORES),
                ag2_o[:, 0:512].rearrange("c (k j) -> k c j", k=4))
            gbot = bigpool.tile([4, 1024], F32)
            nc.gpsimd.dma_start(
                gbot[:].rearrange("k (c j) -> k c j", c=N_CORES),
                ag2_o[:, 512:1024].rearrange("c (k j) -> k c j", k=4))

            gt8 = bigpool.tile([4, 8], F32)
            nc.vector.max(gt8[:], gtop[:])
            gb8 = bigpool.tile([4, 8], F32)
            nc.vector.max(gb8[:], gbot[:])

            # loss = (sum softplus(-top5) + sum softplus(bot5)) / 40
            # bot5 = -gb8[:, :5]; softplus(bot5) = softplus(-gb8) -> same form
            lt = bigpool.tile([4, 2 * K_TOP], F32)
            nc.vector.tensor_copy(lt[:, 0:K_TOP], gt8[:, 0:K_TOP])
            nc.vector.tensor_copy(lt[:, K_TOP:2 * K_TOP], gb8[:, 0:K_TOP])
            sp = bigpool.tile([4, 2 * K_TOP], F32)
            nc.scalar.activation(sp[:], lt[:], AF.Exp, scale=-1.0)
            nc.vector.tensor_scalar_add(sp[:], sp[:], 1.0)
            ls4 = bigpool.tile([4, 1], F32)
            nc.scalar.activation(sp[:], sp[:], AF.Ln, accum_out=ls4[:])
            if DEBUG_OUT:
                nc.sync.dma_start(dbg["d_scores"][:], scores_sb[:])
                nc.sync.dma_start(dbg["d_probs"][:], probs[:])
                nc.sync.dma_start(dbg["d_top8"][:], top8[:])
                nc.sync.dma_start(dbg["d_bot8"][:], bot8n[:])
                nc.sync.dma_start(dbg["d_cand"][:], cand[:])
                nc.gpsimd.dma_start(dbg["d_ag2"][:], ag2_o[:])
                nc.sync.dma_start(dbg["d_gtop"][:], gtop[:])
                nc.sync.dma_start(dbg["d_gbot"][:], gbot[:])
                nc.sync.dma_start(dbg["d_gt8"][:], gt8[:])
                nc.sync.dma_start(dbg["d_gb8"][:], gb8[:])
                nc.sync.dma_start(dbg["d_lt"][:], lt[:])
                nc.sync.dma_start(dbg["d_sp"][:], sp[:])
                nc.sync.dma_start(dbg["d_ls4"][:], ls4[:])
                nc.sync.dma_start(dbg["d_allst"][:], allst[:])
            lsum = bigpool.tile([1, 1], F32)
            nc.gpsimd.tensor_reduce(lsum[:], ls4[:], axis=AX.C,
                                    op=mybir.AluOpType.add)
            loss_sb = bigpool.tile([1, 1], F32)
            nc.vector.tensor_scalar_mul(loss_sb[:], lsum[:],
                                        1.0 / (2 * N_SUB * K_TOP))
            nc.sync.dma_start(loss_out[:], loss_sb[:])

    nc.compile()
    return nc


_NC_CACHE = {}


def _get_nc():
    key = (str(MAIN_MM_DT), DEBUG_OUT)
    if key not in _NC_CACHE:
        _NC_CACHE[key] = _build()
    return _NC_CACHE[key]


def _host_prep(x, text_features, conv_w, conv_b, wa, ba, wt, bt):
    """Build per-core input maps (layout transforms only, no math)."""
    x = np.asarray(x, dtype=np.float32)
    text_features = np.asarray(text_features, dtype=np.float32)
    conv_w = np.asarray(conv_w, dtype=np.float32)
    conv_b = np.asarray(conv_b, dtype=np.float32)
    wa = np.asarray(wa, dtype=np.float32)
    ba = np.asarray(ba, dtype=np.float32)
    wt = np.asarray(wt, dtype=np.float32)
    bt = np.asarray(bt, dtype=np.float32)

    xp = np.zeros((H + 2, W + 2), np.float32)
    xp[1:H + 1, 1:W + 1] = x[0, 0]
    imcol = np.empty((10, KTOT), np.float32)
    t = 0
    for dy in range(3):
        for dx in range(3):
            imcol[t] = xp[dy:dy + H, dx:dx + W].ravel()
            t += 1
    imcol[9] = 1.0

    wc = np.empty((10, 16), np.float32)
    wc[0:9] = conv_w.reshape(16, 9).T
    wc[9] = conv_b

    waT = np.ascontiguousarray(wa.T)              # (321536, 1024)
    wtT = np.ascontiguousarray(wt.T)              # (3072, 256)
    ba2 = np.ascontiguousarray(ba.reshape(1, AUDIO_DIM))
    btT = np.ascontiguousarray(bt.reshape(2, 128).T)  # (128, 2)

    in_maps = []
    for c in range(N_CORES):
        in_maps.append({
            "imcol": np.ascontiguousarray(imcol[:, c * KSH:(c + 1) * KSH]),
            "wc": wc,
            "waT": waT[c * KSH:(c + 1) * KSH, :],
            "textT": np.ascontiguousarray(
                text_features[c * NLOC:(c + 1) * NLOC, :].T),
            "wtT": wtT,
            "ba2": ba2,
            "btT": btT,
        })
    return in_maps


def kernel(x, text_features, conv_w, conv_b, wa, ba, wt, bt, epoch=0, **_):
    nc = _get_nc()
    in_maps = _host_prep(x, text_features, conv_w, conv_b, wa, ba, wt, bt)
    res = bass_utils.run_bass_kernel_spmd(
        nc, in_maps, core_ids=list(range(N_CORES)))
    audio = np.asarray(res.results[0]["audio_out"])
    temb = np.concatenate(
        [np.asarray(res.results[c]["temb_out"]).T for c in range(N_CORES)],
        axis=0)
    loss = np.float32(np.asarray(res.results[0]["loss_out"])[0, 0])
    return audio, temb, loss


# revision 9
# speedup vs baseline: 1.2987x; 1.2987x over previous
"""Trainium2 Bass kernel for nn_ModalityAlignmentModel_2 (topk_masking).

Computation (see reference):
  h = relu(conv2d(x, conv_w) + conv_b)           (1,16,628,512) -> (16, 321536)
  audio = h @ wa.T + ba                          (16, 1024)
  temb  = text_features @ wt.T + bt              (8192, 256)
  scores[k] = audio[:, 256k:256k+256] @ temb.T   (4, 16, 8192)
  probs = softmax(scores, -1); flat per k; loss from top5/bot5 probs.

Sharding (8 cores):
  - Contraction dim of the big matmul (321536) is sharded: core c computes a
    partial (16, 1024) audio embedding from its 40192-slice of h and of wa^T,
    then AllReduce(add).  wa is pre-transposed on host so the contraction dim
    lands on SBUF partitions with fully contiguous DMA.
  - text_features batch dim (8192) is data-parallel: core c computes
    temb[c*1024:(c+1)*1024] (as temb^T (256, 1024)); host concatenates.
  - conv is computed per-core for its own 40192 spatial slice via a host-built
    im2col (10 x 40192, row 9 = ones to fold conv_b into the matmul).
  - softmax over the full 8192 via a tiny AllGather of per-core (max, sum)
    stats; global top-5/bot-5 via per-row top-8 (vector.max), an AllGather of
    per-core candidates, and a final top-8 per sub-embedding.
"""
import sys

import numpy as np

sys.path.insert(0, "/opt/trn_rl_repo")

import concourse.bacc as bacc
import concourse.bass as bass
import concourse.mybir as mybir
import concourse.tile as tile
from concourse import bass_utils, masks

N_CORES = 8
AUDIO_DIM = 1024
TEXT_DIM = 256
N_SUB = 4
K_TOP = 5
H, W = 628, 512
KTOT = H * W            # 321536
KSH = KTOT // N_CORES   # 40192 contraction elements per core
NCHUNK = KSH // 128     # 314 k-chunks of 128
NPAIR = NCHUNK // 2     # 157 pairs (1MB wa DMA each)
NLOC = 8192 // N_CORES  # 1024 text rows per core
TD = 3072               # text feature dim
F32 = mybir.dt.float32
F32R = mybir.dt.float32r
AF = mybir.ActivationFunctionType
AX = mybir.AxisListType

# pairs at which a text d-chunk (of 24) is processed, spread over the wa loop
_TEXT_AT = {t * 6 + 3: t for t in range(24)}

# dtype used for the big wa matmul inputs (f32r = 4x faster PE, same bits)
MAIN_MM_DT = F32
DEBUG_OUT = False
CONV_PREFETCH = 2  # pairs of conv lookahead


def _build():
    nc = bacc.Bacc("TRN2", target_bir_lowering=False, debug=False,
                   enable_asserts=True, num_devices=N_CORES)

    imcol_in = nc.dram_tensor("imcol", [10, KSH], F32, kind="ExternalInput")
    wc_in = nc.dram_tensor("wc", [10, 16], F32, kind="ExternalInput")
    waT_in = nc.dram_tensor("waT", [KSH, AUDIO_DIM], MAIN_MM_DT, kind="ExternalInput")
    textT_in = nc.dram_tensor("textT", [TD, NLOC], F32, kind="ExternalInput")
    wtT_in = nc.dram_tensor("wtT", [TD, TEXT_DIM], F32, kind="ExternalInput")
    ba_in = nc.dram_tensor("ba2", [1, AUDIO_DIM], F32, kind="ExternalInput")
    btT_in = nc.dram_tensor("btT", [128, 2], F32, kind="ExternalInput")

    audio_out = nc.dram_tensor("audio_out", [16, AUDIO_DIM], F32, kind="ExternalOutput")
    temb_out = nc.dram_tensor("temb_out", [TEXT_DIM, NLOC], F32, kind="ExternalOutput")
    loss_out = nc.dram_tensor("loss_out", [1, 1], F32, kind="ExternalOutput")
    dbg = {}
    if DEBUG_OUT:
        for nm, shp in [("d_scores", [128, NLOC]), ("d_probs", [128, NLOC]),
                        ("d_top8", [128, 8]), ("d_bot8", [128, 8]),
                        ("d_cand", [1, 1024]), ("d_ag2", [N_CORES, 1024]),
                        ("d_gtop", [4, 1024]), ("d_gbot", [4, 1024]),
                        ("d_gt8", [4, 8]), ("d_gb8", [4, 8]),
                        ("d_lt", [4, 10]), ("d_sp", [4, 10]),
                        ("d_ls4", [4, 1]), ("d_allst", [128, 16])]:
            dbg[nm] = nc.dram_tensor(nm, shp, F32, kind="ExternalOutput")

    with tile.TileContext(nc) as tc:
        with tc.tile_pool(name="const", bufs=1) as cpool, \
             tc.tile_pool(name="ic", bufs=3) as icpool, \
             tc.tile_pool(name="wa", bufs=6) as wapool, \
             tc.tile_pool(name="ht", bufs=2 * (CONV_PREFETCH + 2)) as htpool, \
             tc.tile_pool(name="tx", bufs=3) as txpool, \
             tc.tile_pool(name="big", bufs=1) as bigpool, \
             tc.tile_pool(name="dram", bufs=1, space="DRAM") as dram, \
             tc.tile_pool(name="pa", bufs=1, space="PSUM") as pa, \
             tc.tile_pool(name="pc", bufs=2, space="PSUM") as pc:

            # ---- constants ----
            wc_sb = cpool.tile([10, 16], F32)
            nc.gpsimd.dma_start(wc_sb[:], wc_in[:])
            wtT_sb = cpool.tile([128, 24 * TEXT_DIM], F32)
            nc.gpsimd.dma_start(
                wtT_sb[:].rearrange("p (i o) -> p i o", i=24),
                wtT_in[:].rearrange("(i p) o -> p i o", p=128))
            btT_sb = cpool.tile([128, 2], F32)
            nc.gpsimd.dma_start(btT_sb[:], btT_in[:])
            ba_sb = cpool.tile([1, AUDIO_DIM], F32)
            nc.gpsimd.dma_start(ba_sb[:], ba_in[:])
            ident = cpool.tile([16, 16], F32)
            masks.make_identity(nc, ident[:])
            ones16 = cpool.tile([1, 16], F32)
            nc.vector.memset(ones16[:], 1.0)

            temb_sb = [bigpool.tile([128, NLOC], F32, tag=f"temb{j}", name=f"temb{j}")
                       for j in range(2)]

            audio_ps = pa.tile([16, AUDIO_DIM], F32)

            ic_tiles = {}  # block -> tile
            ht_tiles = {}  # chunk -> tile

            def load_ic_block(b):
                cols = min(2048, KSH - b * 2048)
                t = icpool.tile([10, 2048], F32)
                nc.gpsimd.dma_start(t[0:10, 0:cols],
                                    imcol_in[:, b * 2048:b * 2048 + cols])
                ic_tiles[b] = t

            def conv_chunk(i):
                """im2col matmul for k-chunk i -> ht_tiles[i] ([128,16] sbuf)."""
                b, off = divmod(i * 128, 2048)
                if b not in ic_tiles:
                    load_ic_block(b)
                cps = pc.tile([128, 16], F32, tag="small")
                nc.tensor.matmul(cps[:], ic_tiles[b][0:10, off:off + 128],
                                 wc_sb[:], start=True, stop=True)
                ht = htpool.tile([128, 16], MAIN_MM_DT)
                nc.scalar.activation(ht[:], cps[:], AF.Relu)
                ht_tiles[i] = ht

            with tc.tile_pool(name="pt", bufs=1, space="PSUM") as pt:
                text_ps = [pt.tile([128, 512], F32, tag=f"pt{j}{t}", name=f"pt{j}{t}")
                           for j in range(2) for t in range(2)]

                def text_chunk(ti):
                    tx = txpool.tile([128, NLOC], F32)
                    nc.gpsimd.dma_start(tx[:], textT_in[ti * 128:(ti + 1) * 128, :])
                    for j in range(2):
                        for t in range(2):
                            nc.tensor.matmul(
                                text_ps[2 * j + t][:],
                                wtT_sb[:, ti * TEXT_DIM + j * 128:
                                       ti * TEXT_DIM + j * 128 + 128],
                                tx[:, t * 512:(t + 1) * 512],
                                start=(ti == 0), stop=(ti == 23))

                # conv prologue
                for i in range(2 * CONV_PREFETCH):
                    conv_chunk(i)

                # ---- main loop: stream waT, accumulate audio partial ----
                for p in range(NPAIR):
                    wa_t = wapool.tile([128, 2048], MAIN_MM_DT)
                    eng = nc.sync if (p % 2 == 0) else nc.scalar
                    eng.dma_start(
                        wa_t[:].rearrange("p (c o) -> p c o", c=2),
                        waT_in[p * 256:(p + 1) * 256, :]
                        .rearrange("(c p) o -> p c o", p=128))
                    # conv lookahead
                    base = (p + CONV_PREFETCH) * 2
                    for s in (0, 1):
                        if base + s < NCHUNK:
                            conv_chunk(base + s)
                    # audio matmuls for this pair
                    for s in (0, 1):
                        i = 2 * p + s
                        ht = ht_tiles.pop(i)
                        for half in (0, 1):
                            nc.tensor.matmul(
                                audio_ps[:, half * 512:(half + 1) * 512],
                                ht[:],
                                wa_t[:, s * 1024 + half * 512:
                                     s * 1024 + (half + 1) * 512],
                                start=(i == 0), stop=(i == NCHUNK - 1))
                    if p in _TEXT_AT:
                        text_chunk(_TEXT_AT[p])

                # ---- text epilogue: bias + copy out ----
                for j in range(2):
                    for t in range(2):
                        nc.scalar.activation(
                            temb_sb[j][:, t * 512:(t + 1) * 512],
                            text_ps[2 * j + t][:], AF.Identity,
                            bias=btT_sb[:, j:j + 1])
                    nc.sync.dma_start(
                        temb_out[j * 128:(j + 1) * 128, :], temb_sb[j][:])

            # ---- audio epilogue: AllReduce + bias + transpose ----
            audio_sb = bigpool.tile([16, AUDIO_DIM], F32)
            nc.vector.tensor_copy(audio_sb[:], audio_ps[:])
            ar_i = dram.tile([16, AUDIO_DIM], F32)
            ar_o = dram.tile([16, AUDIO_DIM], F32)
            nc.gpsimd.dma_start(ar_i[:], audio_sb[:])
            nc.gpsimd.collective_compute(
                "AllReduce", mybir.AluOpType.add,
                replica_groups=[list(range(N_CORES))],
                ins=[ar_i.opt()], outs=[ar_o.opt()])
            audio_red = bigpool.tile([16, AUDIO_DIM], F32)
            nc.gpsimd.dma_start(audio_red[:], ar_o[:])

            with tc.tile_pool(name="pe", bufs=2, space="PSUM") as pe:
                bb_ps = pe.tile([16, AUDIO_DIM], F32, tag="wide")
                for half in (0, 1):
                    nc.tensor.matmul(bb_ps[:, half * 512:(half + 1) * 512],
                                     ones16[:],
                                     ba_sb[:, half * 512:(half + 1) * 512],
                                     start=True, stop=True)
                audio_f = bigpool.tile([16, AUDIO_DIM], F32)
                nc.vector.tensor_add(audio_f[:], audio_red[:], bb_ps[:])
                nc.sync.dma_start(audio_out[:], audio_f[:])

                audioT = bigpool.tile([128, 128], F32)
                for j in range(8):
                    tp = pc.tile([128, 16], F32, tag="small")
                    nc.tensor.transpose(tp[:], audio_f[:, j * 128:(j + 1) * 128],
                                        ident[:])
                    nc.vector.tensor_copy(audioT[:, j * 16:(j + 1) * 16], tp[:])

                # ---- scores: (4, 16, 1024_local) ----
                # row layout: sub k occupies partitions [k*32, k*32+16)
                # (32-aligned engine base-partition requirement); rest zeroed.
                scores_sb = bigpool.tile([128, NLOC], F32)
                nc.vector.memset(scores_sb[:], 0.0)
                for k in range(N_SUB):
                    sc = pe.tile([16, NLOC], F32, tag="wide")
                    for h in (0, 1):
                        for t in (0, 1):
                            nc.tensor.matmul(
                                sc[:, t * 512:(t + 1) * 512],
                                audioT[:, (2 * k + h) * 16:(2 * k + h + 1) * 16],
                                temb_sb[h][:, t * 512:(t + 1) * 512],
                                start=(h == 0), stop=(h == 1))
                    nc.vector.tensor_copy(
                        scores_sb[k * 32:k * 32 + 16, :], sc[:])

            # ---- softmax with global stats ----
            m_loc = bigpool.tile([128, 1], F32)
            nc.vector.reduce_max(m_loc[:], scores_sb[:], axis=AX.X)
            negm = bigpool.tile([128, 1], F32)
            nc.vector.tensor_scalar_mul(negm[:], m_loc[:], -1.0)
            e_loc = bigpool.tile([128, NLOC], F32)
            z_loc = bigpool.tile([128, 1], F32)
            nc.scalar.activation(e_loc[:], scores_sb[:], AF.Exp,
                                 bias=negm[:], accum_out=z_loc[:])

            stats = bigpool.tile([128, 2], F32)
            nc.vector.tensor_copy(stats[:, 0:1], m_loc[:])
            nc.vector.tensor_copy(stats[:, 1:2], z_loc[:])
            ag1_i = dram.tile([128, 2], F32)
            ag1_o = dram.tile([128 * N_CORES, 2], F32)
            nc.gpsimd.dma_start(ag1_i[:], stats[:])
            nc.gpsimd.collective_compute(
                "AllGather", mybir.AluOpType.bypass,
                replica_groups=[list(range(N_CORES))],
                ins=[ag1_i.opt()], outs=[ag1_o.opt()])
            allst = bigpool.tile([128, 16], F32)
            nc.gpsimd.dma_start(
                allst[:].rearrange("r (c v) -> r c v", c=N_CORES),
                ag1_o[:].rearrange("(c r) v -> r c v", c=N_CORES))

            allst_v = allst[:].rearrange("r (c v) -> r c v", c=N_CORES)
            m_cols = allst_v[:, :, 0]
            z_cols = allst_v[:, :, 1]
            gmax = bigpool.tile([128, 1], F32)
            nc.vector.reduce_max(gmax[:], m_cols, axis=AX.X)
            neggm = bigpool.tile([128, 1], F32)
            nc.vector.tensor_scalar_mul(neggm[:], gmax[:], -1.0)
            t8 = bigpool.tile([128, N_CORES], F32)
            nc.scalar.activation(t8[:], m_cols, AF.Exp, bias=neggm[:])
            tz = bigpool.tile([128, N_CORES], F32)
            nc.vector.tensor_mul(tz[:], t8[:], z_cols)
            zg = bigpool.tile([128, 1], F32)
            nc.vector.reduce_sum(zg[:], tz[:], axis=AX.X)
            rz = bigpool.tile([128, 1], F32)
            nc.vector.reciprocal(rz[:], zg[:])

            probs = bigpool.tile([128, NLOC], F32)
            nc.scalar.activation(probs[:], scores_sb[:], AF.Exp, bias=neggm[:])
            nc.vector.tensor_scalar_mul(probs[:], probs[:], rz[:])
            negp = bigpool.tile([128, NLOC], F32)
            nc.vector.tensor_scalar_mul(negp[:], probs[:], -1.0)

            # ---- local top-8 / bot-8 per (k, b) row, then AllGather ----
            top8 = bigpool.tile([128, 8], F32)
            nc.vector.max(top8[:], probs[:])
            bot8n = bigpool.tile([128, 8], F32)
            nc.vector.max(bot8n[:], negp[:])
            cand = bigpool.tile([1, 1024], F32)
            for k in range(N_SUB):
                nc.sync.dma_start(cand[0:1, k * 128:(k + 1) * 128],
                                  top8[k * 32:k * 32 + 16, :])
                nc.sync.dma_start(cand[0:1, 512 + k * 128:512 + (k + 1) * 128],
                                  bot8n[k * 32:k * 32 + 16, :])
            ag2_i = dram.tile([1, 1024], F32)
            ag2_o = dram.tile([N_CORES, 1024], F32)
            nc.gpsimd.dma_start(ag2_i[:], cand[:])
            nc.gpsimd.collective_compute(
                "AllGather", mybir.AluOpType.bypass,
                replica_groups=[list(range(N_CORES))],
                ins=[ag2_i.opt()], outs=[ag2_o.opt()])
            gtop = bigpool.tile([4, 1024], F32)
            nc.gpsimd.dma_start(
                gtop[:].rearrange("k (c j) -> k c j", c=N_CORES),
                ag2_o[:, 0:512].rearrange("c (k j) -> k c j", k=4))
            gbot = bigpool.tile([4, 1024], F32)
            nc.gpsimd.dma_start(
                gbot[:].rearrange("k (c j) -> k c j", c=N_CORES),
                ag2_o[:, 512:1024].rearrange("c (k j) -> k c j", k=4))

            gt8 = bigpool.tile([4, 8], F32)
            nc.vector.max(gt8[:], gtop[:])
            gb8 = bigpool.tile([4, 8], F32)
            nc.vector.max(gb8[:], gbot[:])

            # loss = (sum softplus(-top5) + sum softplus(bot5)) / 40
            # bot5 = -gb8[:, :5]; softplus(bot5) = softplus(-gb8) -> same form
            lt = bigpool.tile([4, 2 * K_TOP], F32)
            nc.vector.tensor_copy(lt[:, 0:K_TOP], gt8[:, 0:K_TOP])
            nc.vector.tensor_copy(lt[:, K_TOP:2 * K_TOP], gb8[:, 0:K_TOP])
            sp = bigpool.tile([4, 2 * K_TOP], F32)
            nc.scalar.activation(sp[:], lt[:], AF.Exp, scale=-1.0)
            nc.vector.tensor_scalar_add(sp[:], sp[:], 1.0)
            ls4 = bigpool.tile([4, 1], F32)
            nc.scalar.activation(sp[:], sp[:], AF.Ln, accum_out=ls4[:])
            if DEBUG_OUT:
                nc.sync.dma_start(dbg["d_scores"][:], scores_sb[:])
                nc.sync.dma_start(dbg["d_probs"][:], probs[:])
                nc.sync.dma_start(dbg["d_top8"][:], top8[:])
                nc.sync.dma_start(dbg["d_bot8"][:], bot8n[:])
                nc.sync.dma_start(dbg["d_cand"][:], cand[:])
                nc.gpsimd.dma_start(dbg["d_ag2"][:], ag2_o[:])
                nc.sync.dma_start(dbg["d_gtop"][:], gtop[:])
                nc.sync.dma_start(dbg["d_gbot"][:], gbot[:])
                nc.sync.dma_start(dbg["d_gt8"][:], gt8[:])
                nc.sync.dma_start(dbg["d_gb8"][:], gb8[:])
                nc.sync.dma_start(dbg["d_lt"][:], lt[:])
                nc.sync.dma_start(dbg["d_sp"][:], sp[:])
                nc.sync.dma_start(dbg["d_ls4"][:], ls4[:])
                nc.sync.dma_start(dbg["d_allst"][:], allst[:])
            lsum = bigpool.tile([1, 1], F32)
            nc.gpsimd.tensor_reduce(lsum[:], ls4[:], axis=AX.C,
                                    op=mybir.AluOpType.add)
            loss_sb = bigpool.tile([1, 1], F32)
            nc.vector.tensor_scalar_mul(loss_sb[:], lsum[:],
                                        1.0 / (2 * N_SUB * K_TOP))
            nc.sync.dma_start(loss_out[:], loss_sb[:])

    nc.compile()
    return nc


_NC_CACHE = {}


def _get_nc():
    key = (str(MAIN_MM_DT), DEBUG_OUT)
    if key not in _NC_CACHE:
        _NC_CACHE[key] = _build()
    return _NC_CACHE[key]


def _host_prep(x, text_features, conv_w, conv_b, wa, ba, wt, bt):
    """Build per-core input maps (layout transforms only, no math)."""
    x = np.asarray(x, dtype=np.float32)
    text_features = np.asarray(text_features, dtype=np.float32)
    conv_w = np.asarray(conv_w, dtype=np.float32)
    conv_b = np.asarray(conv_b, dtype=np.float32)
    wa = np.asarray(wa, dtype=np.float32)
    ba = np.asarray(ba, dtype=np.float32)
    wt = np.asarray(wt, dtype=np.float32)
    bt = np.asarray(bt, dtype=np.float32)

    xp = np.zeros((H + 2, W + 2), np.float32)
    xp[1:H + 1, 1:W + 1] = x[0, 0]
    imcol = np.empty((10, KTOT), np.float32)
    t = 0
    for dy in range(3):
        for dx in range(3):
            imcol[t] = xp[dy:dy + H, dx:dx + W].ravel()
            t += 1
    imcol[9] = 1.0

    wc = np.empty((10, 16), np.float32)
    wc[0:9] = conv_w.reshape(16, 9).T
    wc[9] = conv_b

    waT = np.ascontiguousarray(wa.T)              # (321536, 1024)
    wtT = np.ascontiguousarray(wt.T)              # (3072, 256)
    ba2 = np.ascontiguousarray(ba.reshape(1, AUDIO_DIM))
    btT = np.ascontiguousarray(bt.reshape(2, 128).T)  # (128, 2)

    in_maps = []
    for c in range(N_CORES):
        in_maps.append({
            "imcol": np.ascontiguousarray(imcol[:, c * KSH:(c + 1) * KSH]),
            "wc": wc,
            "waT": waT[c * KSH:(c + 1) * KSH, :],
            "textT": np.ascontiguousarray(
                text_features[c * NLOC:(c + 1) * NLOC, :].T),
            "wtT": wtT,
            "ba2": ba2,
            "btT": btT,
        })
    return in_maps


def kernel(x, text_features, conv_w, conv_b, wa, ba, wt, bt, epoch=0, **_):
    nc = _get_nc()
    in_maps = _host_prep(x, text_features, conv_w, conv_b, wa, ba, wt, bt)
    res = bass_utils.run_bass_kernel_spmd(
        nc, in_maps, core_ids=list(range(N_CORES)))
    audio = np.asarray(res.results[0]["audio_out"])
    temb = np.concatenate(
        [np.asarray(res.results[c]["temb_out"]).T for c in range(N_CORES)],
        axis=0)
    loss = np.float32(np.asarray(res.results[0]["loss_out"])[0, 0])
    return audio, temb, loss


# revision 10
# speedup vs baseline: 1.3004x; 1.0013x over previous
"""Trainium2 Bass kernel for nn_ModalityAlignmentModel_2 (topk_masking).

Computation (see reference):
  h = relu(conv2d(x, conv_w) + conv_b)           (1,16,628,512) -> (16, 321536)
  audio = h @ wa.T + ba                          (16, 1024)
  temb  = text_features @ wt.T + bt              (8192, 256)
  scores[k] = audio[:, 256k:256k+256] @ temb.T   (4, 16, 8192)
  probs = softmax(scores, -1); flat per k; loss from top5/bot5 probs.

Sharding (8 cores):
  - Contraction dim of the big matmul (321536) is sharded: core c computes a
    partial (16, 1024) audio embedding from its 40192-slice of h and of wa^T,
    then AllReduce(add).  wa is pre-transposed on host so the contraction dim
    lands on SBUF partitions with fully contiguous DMA.
  - text_features batch dim (8192) is data-parallel: core c computes
    temb[c*1024:(c+1)*1024] (as temb^T (256, 1024)); host concatenates.
  - conv is computed per-core for its own 40192 spatial slice via a host-built
    im2col (10 x 40192, row 9 = ones to fold conv_b into the matmul).
  - softmax over the full 8192 via a tiny AllGather of per-core (max, sum)
    stats; global top-5/bot-5 via per-row top-8 (vector.max), an AllGather of
    per-core candidates, and a final top-8 per sub-embedding.
"""
import sys

import numpy as np

sys.path.insert(0, "/opt/trn_rl_repo")

import concourse.bacc as bacc
import concourse.bass as bass
import concourse.mybir as mybir
import concourse.tile as tile
from concourse import bass_utils, masks

N_CORES = 8
AUDIO_DIM = 1024
TEXT_DIM = 256
N_SUB = 4
K_TOP = 5
H, W = 628, 512
KTOT = H * W            # 321536
KSH = KTOT // N_CORES   # 40192 contraction elements per core
NCHUNK = KSH // 128     # 314 k-chunks of 128
NPAIR = NCHUNK // 2     # 157 pairs (1MB wa DMA each)
NLOC = 8192 // N_CORES  # 1024 text rows per core
TD = 3072               # text feature dim
F32 = mybir.dt.float32
F32R = mybir.dt.float32r
AF = mybir.ActivationFunctionType
AX = mybir.AxisListType

# pairs at which a text d-chunk (of 24) is processed, spread over the wa loop
_TEXT_AT = {t * 6 + 3: t for t in range(24)}

# dtype used for the big wa matmul inputs (f32r = 4x faster PE, same bits)
MAIN_MM_DT = F32
DEBUG_OUT = False
CONV_PREFETCH = 2  # pairs of conv lookahead


def _build():
    nc = bacc.Bacc("TRN2", target_bir_lowering=False, debug=False,
                   enable_asserts=True, num_devices=N_CORES)

    imcol_in = nc.dram_tensor("imcol", [10, KSH], MAIN_MM_DT, kind="ExternalInput")
    wc_in = nc.dram_tensor("wc", [10, 16], MAIN_MM_DT, kind="ExternalInput")
    waT_in = nc.dram_tensor("waT", [KSH, AUDIO_DIM], MAIN_MM_DT, kind="ExternalInput")
    textT_in = nc.dram_tensor("textT", [TD, NLOC], MAIN_MM_DT, kind="ExternalInput")
    wtT_in = nc.dram_tensor("wtT", [TD, TEXT_DIM], MAIN_MM_DT, kind="ExternalInput")
    ba_in = nc.dram_tensor("ba2", [1, AUDIO_DIM], F32, kind="ExternalInput")
    btT_in = nc.dram_tensor("btT", [128, 2], F32, kind="ExternalInput")

    audio_out = nc.dram_tensor("audio_out", [16, AUDIO_DIM], F32, kind="ExternalOutput")
    temb_out = nc.dram_tensor("temb_out", [TEXT_DIM, NLOC], F32, kind="ExternalOutput")
    loss_out = nc.dram_tensor("loss_out", [1, 1], F32, kind="ExternalOutput")
    dbg = {}
    if DEBUG_OUT:
        for nm, shp in [("d_scores", [128, NLOC]), ("d_probs", [128, NLOC]),
                        ("d_top8", [128, 8]), ("d_bot8", [128, 8]),
                        ("d_cand", [1, 1024]), ("d_ag2", [N_CORES, 1024]),
                        ("d_gtop", [4, 1024]), ("d_gbot", [4, 1024]),
                        ("d_gt8", [4, 8]), ("d_gb8", [4, 8]),
                        ("d_lt", [4, 10]), ("d_sp", [4, 10]),
                        ("d_ls4", [4, 1]), ("d_allst", [128, 16])]:
            dbg[nm] = nc.dram_tensor(nm, shp, F32, kind="ExternalOutput")

    with tile.TileContext(nc) as tc:
        with tc.tile_pool(name="const", bufs=1) as cpool, \
             tc.tile_pool(name="ic", bufs=3) as icpool, \
             tc.tile_pool(name="wa", bufs=6) as wapool, \
             tc.tile_pool(name="ht", bufs=2 * (CONV_PREFETCH + 2)) as htpool, \
             tc.tile_pool(name="tx", bufs=3) as txpool, \
             tc.tile_pool(name="big", bufs=1) as bigpool, \
             tc.tile_pool(name="dram", bufs=1, space="DRAM") as dram, \
             tc.tile_pool(name="pa", bufs=1, space="PSUM") as pa, \
             tc.tile_pool(name="pc", bufs=2, space="PSUM") as pc:

            # ---- constants ----
            wc_sb = cpool.tile([10, 16], MAIN_MM_DT)
            nc.gpsimd.dma_start(wc_sb[:], wc_in[:])
            wtT_sb = cpool.tile([128, 24 * TEXT_DIM], MAIN_MM_DT)
            nc.gpsimd.dma_start(
                wtT_sb[:].rearrange("p (i o) -> p i o", i=24),
                wtT_in[:].rearrange("(i p) o -> p i o", p=128))
            btT_sb = cpool.tile([128, 2], F32)
            nc.gpsimd.dma_start(btT_sb[:], btT_in[:])
            ba_sb = cpool.tile([1, AUDIO_DIM], F32)
            nc.gpsimd.dma_start(ba_sb[:], ba_in[:])
            ident = cpool.tile([16, 16], F32)
            masks.make_identity(nc, ident[:])
            ones16 = cpool.tile([1, 16], F32)
            nc.vector.memset(ones16[:], 1.0)

            temb_sb = [bigpool.tile([128, NLOC], F32, tag=f"temb{j}", name=f"temb{j}")
                       for j in range(2)]

            audio_ps = pa.tile([16, AUDIO_DIM], F32)

            ic_tiles = {}  # block -> tile
            ht_tiles = {}  # chunk -> tile

            def load_ic_block(b):
                cols = min(2048, KSH - b * 2048)
                t = icpool.tile([10, 2048], MAIN_MM_DT)
                nc.gpsimd.dma_start(t[0:10, 0:cols],
                                    imcol_in[:, b * 2048:b * 2048 + cols])
                ic_tiles[b] = t

            def conv_chunk(i):
                """im2col matmul for k-chunk i -> ht_tiles[i] ([128,16] sbuf)."""
                b, off = divmod(i * 128, 2048)
                if b not in ic_tiles:
                    load_ic_block(b)
                cps = pc.tile([128, 16], F32, tag="small")
                nc.tensor.matmul(cps[:], ic_tiles[b][0:10, off:off + 128],
                                 wc_sb[:], start=True, stop=True)
                ht = htpool.tile([128, 16], MAIN_MM_DT)
                nc.scalar.activation(ht[:], cps[:], AF.Relu)
                ht_tiles[i] = ht

            with tc.tile_pool(name="pt", bufs=1, space="PSUM") as pt:
                text_ps = [pt.tile([128, 512], F32, tag=f"pt{j}{t}", name=f"pt{j}{t}")
                           for j in range(2) for t in range(2)]

                def text_chunk(ti):
                    tx = txpool.tile([128, NLOC], MAIN_MM_DT)
                    nc.gpsimd.dma_start(tx[:], textT_in[ti * 128:(ti + 1) * 128, :])
                    for j in range(2):
                        for t in range(2):
                            nc.tensor.matmul(
                                text_ps[2 * j + t][:],
                                wtT_sb[:, ti * TEXT_DIM + j * 128:
                                       ti * TEXT_DIM + j * 128 + 128],
                                tx[:, t * 512:(t + 1) * 512],
                                start=(ti == 0), stop=(ti == 23))

                # conv prologue
                for i in range(2 * CONV_PREFETCH):
                    conv_chunk(i)

                # ---- main loop: stream waT, accumulate audio partial ----
                for p in range(NPAIR):
                    wa_t = wapool.tile([128, 2048], MAIN_MM_DT)
                    eng = nc.sync if (p % 2 == 0) else nc.scalar
                    eng.dma_start(
                        wa_t[:].rearrange("p (c o) -> p c o", c=2),
                        waT_in[p * 256:(p + 1) * 256, :]
                        .rearrange("(c p) o -> p c o", p=128))
                    # conv lookahead
                    base = (p + CONV_PREFETCH) * 2
                    for s in (0, 1):
                        if base + s < NCHUNK:
                            conv_chunk(base + s)
                    # audio matmuls for this pair
                    for s in (0, 1):
                        i = 2 * p + s
                        ht = ht_tiles.pop(i)
                        for half in (0, 1):
                            nc.tensor.matmul(
                                audio_ps[:, half * 512:(half + 1) * 512],
                                ht[:],
                                wa_t[:, s * 1024 + half * 512:
                                     s * 1024 + (half + 1) * 512],
                                start=(i == 0), stop=(i == NCHUNK - 1))
                    if p in _TEXT_AT:
                        text_chunk(_TEXT_AT[p])

                # ---- text epilogue: bias + copy out ----
                for j in range(2):
                    for t in range(2):
                        nc.scalar.activation(
                            temb_sb[j][:, t * 512:(t + 1) * 512],
                            text_ps[2 * j + t][:], AF.Identity,
                            bias=btT_sb[:, j:j + 1])
                    nc.sync.dma_start(
                        temb_out[j * 128:(j + 1) * 128, :], temb_sb[j][:])

            # ---- audio epilogue: AllReduce + bias + transpose ----
            audio_sb = bigpool.tile([16, AUDIO_DIM], F32)
            nc.vector.tensor_copy(audio_sb[:], audio_ps[:])
            ar_i = dram.tile([16, AUDIO_DIM], F32)
            ar_o = dram.tile([16, AUDIO_DIM], F32)
            nc.gpsimd.dma_start(ar_i[:], audio_sb[:])
            nc.gpsimd.collective_compute(
                "AllReduce", mybir.AluOpType.add,
                replica_groups=[list(range(N_CORES))],
                ins=[ar_i.opt()], outs=[ar_o.opt()])
            audio_red = bigpool.tile([16, AUDIO_DIM], F32)
            nc.gpsimd.dma_start(audio_red[:], ar_o[:])

            with tc.tile_pool(name="pe", bufs=2, space="PSUM") as pe:
                bb_ps = pe.tile([16, AUDIO_DIM], F32, tag="wide")
                for half in (0, 1):
                    nc.tensor.matmul(bb_ps[:, half * 512:(half + 1) * 512],
                                     ones16[:],
                                     ba_sb[:, half * 512:(half + 1) * 512],
                                     start=True, stop=True)
                audio_f = bigpool.tile([16, AUDIO_DIM], F32)
                nc.vector.tensor_add(audio_f[:], audio_red[:], bb_ps[:])
                nc.sync.dma_start(audio_out[:], audio_f[:])

                audioT = bigpool.tile([128, 128], F32)
                for j in range(8):
                    tp = pc.tile([128, 16], F32, tag="small")
                    nc.tensor.transpose(tp[:], audio_f[:, j * 128:(j + 1) * 128],
                                        ident[:])
                    nc.vector.tensor_copy(audioT[:, j * 16:(j + 1) * 16], tp[:])

                # ---- scores: (4, 16, 1024_local) ----
                # row layout: sub k occupies partitions [k*32, k*32+16)
                # (32-aligned engine base-partition requirement); rest zeroed.
                scores_sb = bigpool.tile([128, NLOC], F32)
                nc.vector.memset(scores_sb[:], 0.0)
                for k in range(N_SUB):
                    sc = pe.tile([16, NLOC], F32, tag="wide")
                    for h in (0, 1):
                        for t in (0, 1):
                            nc.tensor.matmul(
                                sc[:, t * 512:(t + 1) * 512],
                                audioT[:, (2 * k + h) * 16:(2 * k + h + 1) * 16],
                                temb_sb[h][:, t * 512:(t + 1) * 512],
                                start=(h == 0), stop=(h == 1))
                    nc.vector.tensor_copy(
                        scores_sb[k * 32:k * 32 + 16, :], sc[:])

            # ---- softmax with global stats ----
            m_loc = bigpool.tile([128, 1], F32)
            nc.vector.reduce_max(m_loc[:], scores_sb[:], axis=AX.X)
            negm = bigpool.tile([128, 1], F32)
            nc.vector.tensor_scalar_mul(negm[:], m_loc[:], -1.0)
            e_loc = bigpool.tile([128, NLOC], F32)
            z_loc = bigpool.tile([128, 1], F32)
            nc.scalar.activation(e_loc[:], scores_sb[:], AF.Exp,
                                 bias=negm[:], accum_out=z_loc[:])

            stats = bigpool.tile([128, 2], F32)
            nc.vector.tensor_copy(stats[:, 0:1], m_loc[:])
            nc.vector.tensor_copy(stats[:, 1:2], z_loc[:])
            ag1_i = dram.tile([128, 2], F32)
            ag1_o = dram.tile([128 * N_CORES, 2], F32)
            nc.gpsimd.dma_start(ag1_i[:], stats[:])
            nc.gpsimd.collective_compute(
                "AllGather", mybir.AluOpType.bypass,
                replica_groups=[list(range(N_CORES))],
                ins=[ag1_i.opt()], outs=[ag1_o.opt()])
            allst = bigpool.tile([128, 16], F32)
            nc.gpsimd.dma_start(
                allst[:].rearrange("r (c v) -> r c v", c=N_CORES),
                ag1_o[:].rearrange("(c r) v -> r c v", c=N_CORES))

            allst_v = allst[:].rearrange("r (c v) -> r c v", c=N_CORES)
            m_cols = allst_v[:, :, 0]
            z_cols = allst_v[:, :, 1]
            gmax = bigpool.tile([128, 1], F32)
            nc.vector.reduce_max(gmax[:], m_cols, axis=AX.X)
            neggm = bigpool.tile([128, 1], F32)
            nc.vector.tensor_scalar_mul(neggm[:], gmax[:], -1.0)
            t8 = bigpool.tile([128, N_CORES], F32)
            nc.scalar.activation(t8[:], m_cols, AF.Exp, bias=neggm[:])
            tz = bigpool.tile([128, N_CORES], F32)
            nc.vector.tensor_mul(tz[:], t8[:], z_cols)
            zg = bigpool.tile([128, 1], F32)
            nc.vector.reduce_sum(zg[:], tz[:], axis=AX.X)
            rz = bigpool.tile([128, 1], F32)
            nc.vector.reciprocal(rz[:], zg[:])

            probs = bigpool.tile([128, NLOC], F32)
            nc.scalar.activation(probs[:], scores_sb[:], AF.Exp, bias=neggm[:])
            nc.vector.tensor_scalar_mul(probs[:], probs[:], rz[:])
            negp = bigpool.tile([128, NLOC], F32)
            nc.vector.tensor_scalar_mul(negp[:], probs[:], -1.0)

            # ---- local top-8 / bot-8 per (k, b) row, then AllGather ----
            top8 = bigpool.tile([128, 8], F32)
            nc.vector.max(top8[:], probs[:])
            bot8n = bigpool.tile([128, 8], F32)
            nc.vector.max(bot8n[:], negp[:])
            cand = bigpool.tile([1, 1024], F32)
            for k in range(N_SUB):
                nc.sync.dma_start(cand[0:1, k * 128:(k + 1) * 128],
                                  top8[k * 32:k * 32 + 16, :])
                nc.sync.dma_start(cand[0:1, 512 + k * 128:512 + (k + 1) * 128],
                                  bot8n[k * 32:k * 32 + 16, :])
            ag2_i = dram.tile([1, 1024], F32)
            ag2_o = dram.tile([N_CORES, 1024], F32)
            nc.gpsimd.dma_start(ag2_i[:], cand[:])
            nc.gpsimd.collective_compute(
                "AllGather", mybir.AluOpType.bypass,
                replica_groups=[list(range(N_CORES))],
                ins=[ag2_i.opt()], outs=[ag2_o.opt()])
            gtop = bigpool.tile([4, 1024], F32)
            nc.gpsimd.dma_start(
                gtop[:].rearrange("k (c j) -> k c j", c=N_CORES),
                ag2_o[:, 0:512].rearrange("c (k j) -> k c j", k=4))
            gbot = bigpool.tile([4, 1024], F32)
            nc.gpsimd.dma_start(
                gbot[:].rearrange("k (c j) -> k c j", c=N_CORES),
                ag2_o[:, 512:1024].rearrange("c (k j) -> k c j", k=4))

            gt8 = bigpool.tile([4, 8], F32)
            nc.vector.max(gt8[:], gtop[:])
            gb8 = bigpool.tile([4, 8], F32)
            nc.vector.max(gb8[:], gbot[:])

            # loss = (sum softplus(-top5) + sum softplus(bot5)) / 40
            # bot5 = -gb8[:, :5]; softplus(bot5) = softplus(-gb8) -> same form
            lt = bigpool.tile([4, 2 * K_TOP], F32)
            nc.vector.tensor_copy(lt[:, 0:K_TOP], gt8[:, 0:K_TOP])
            nc.vector.tensor_copy(lt[:, K_TOP:2 * K_TOP], gb8[:, 0:K_TOP])
            sp = bigpool.tile([4, 2 * K_TOP], F32)
            nc.scalar.activation(sp[:], lt[:], AF.Exp, scale=-1.0)
            nc.vector.tensor_scalar_add(sp[:], sp[:], 1.0)
            ls4 = bigpool.tile([4, 1], F32)
            nc.scalar.activation(sp[:], sp[:], AF.Ln, accum_out=ls4[:])
            if DEBUG_OUT:
                nc.sync.dma_start(dbg["d_scores"][:], scores_sb[:])
                nc.sync.dma_start(dbg["d_probs"][:], probs[:])
                nc.sync.dma_start(dbg["d_top8"][:], top8[:])
                nc.sync.dma_start(dbg["d_bot8"][:], bot8n[:])
                nc.sync.dma_start(dbg["d_cand"][:], cand[:])
                nc.gpsimd.dma_start(dbg["d_ag2"][:], ag2_o[:])
                nc.sync.dma_start(dbg["d_gtop"][:], gtop[:])
                nc.sync.dma_start(dbg["d_gbot"][:], gbot[:])
                nc.sync.dma_start(dbg["d_gt8"][:], gt8[:])
                nc.sync.dma_start(dbg["d_gb8"][:], gb8[:])
                nc.sync.dma_start(dbg["d_lt"][:], lt[:])
                nc.sync.dma_start(dbg["d_sp"][:], sp[:])
                nc.sync.dma_start(dbg["d_ls4"][:], ls4[:])
                nc.sync.dma_start(dbg["d_allst"][:], allst[:])
            lsum = bigpool.tile([1, 1], F32)
            nc.gpsimd.tensor_reduce(lsum[:], ls4[:], axis=AX.C,
                                    op=mybir.AluOpType.add)
            loss_sb = bigpool.tile([1, 1], F32)
            nc.vector.tensor_scalar_mul(loss_sb[:], lsum[:],
                                        1.0 / (2 * N_SUB * K_TOP))
            nc.sync.dma_start(loss_out[:], loss_sb[:])

    nc.compile()
    return nc


_NC_CACHE = {}


def _get_nc():
    key = (str(MAIN_MM_DT), DEBUG_OUT)
    if key not in _NC_CACHE:
        _NC_CACHE[key] = _build()
    return _NC_CACHE[key]


def _host_prep(x, text_features, conv_w, conv_b, wa, ba, wt, bt):
    """Build per-core input maps (layout transforms only, no math)."""
    x = np.asarray(x, dtype=np.float32)
    text_features = np.asarray(text_features, dtype=np.float32)
    conv_w = np.asarray(conv_w, dtype=np.float32)
    conv_b = np.asarray(conv_b, dtype=np.float32)
    wa = np.asarray(wa, dtype=np.float32)
    ba = np.asarray(ba, dtype=np.float32)
    wt = np.asarray(wt, dtype=np.float32)
    bt = np.asarray(bt, dtype=np.float32)

    xp = np.zeros((H + 2, W + 2), np.float32)
    xp[1:H + 1, 1:W + 1] = x[0, 0]
    imcol = np.empty((10, KTOT), np.float32)
    t = 0
    for dy in range(3):
        for dx in range(3):
            imcol[t] = xp[dy:dy + H, dx:dx + W].ravel()
            t += 1
    imcol[9] = 1.0

    wc = np.empty((10, 16), np.float32)
    wc[0:9] = conv_w.reshape(16, 9).T
    wc[9] = conv_b

    waT = np.ascontiguousarray(wa.T)              # (321536, 1024)
    wtT = np.ascontiguousarray(wt.T)              # (3072, 256)
    ba2 = np.ascontiguousarray(ba.reshape(1, AUDIO_DIM))
    btT = np.ascontiguousarray(bt.reshape(2, 128).T)  # (128, 2)

    in_maps = []
    for c in range(N_CORES):
        in_maps.append({
            "imcol": np.ascontiguousarray(imcol[:, c * KSH:(c + 1) * KSH]),
            "wc": wc,
            "waT": waT[c * KSH:(c + 1) * KSH, :],
            "textT": np.ascontiguousarray(
                text_features[c * NLOC:(c + 1) * NLOC, :].T),
            "wtT": wtT,
            "ba2": ba2,
            "btT": btT,
        })
    return in_maps


def kernel(x, text_features, conv_w, conv_b, wa, ba, wt, bt, epoch=0, **_):
    nc = _get_nc()
    in_maps = _host_prep(x, text_features, conv_w, conv_b, wa, ba, wt, bt)
    res = bass_utils.run_bass_kernel_spmd(
        nc, in_maps, core_ids=list(range(N_CORES)))
    audio = np.asarray(res.results[0]["audio_out"])
    temb = np.concatenate(
        [np.asarray(res.results[c]["temb_out"]).T for c in range(N_CORES)],
        axis=0)
    loss = np.float32(np.asarray(res.results[0]["loss_out"])[0, 0])
    return audio, temb, loss
